# revision 1
# baseline (speedup 1.0000x reference)
"""BarrierNet Trainium2 kernel: 8-core data-parallel Bass/Tile implementation.

Takes full inputs, shards batch across 8 NeuronCores, returns full output.

Layout strategy (per core, S = 131072 samples):
  - obs loaded naturally: partition p of a span holds samples [base+64p, base+64p+64)
    (4KB contiguous per partition -> full DMA efficiency).
  - PE block-transposes [128,128] natural blocks into packed obsT (rows 16*t8+f).
  - MLP on PE in transposed activation layout:
      L1: K=32 matmuls with zero-padded w1 pairs (row strips, tile_position),
      L2: col-tiled K=128 matmuls (4 col strips of w2),
      L3: block-diagonal w3 -> u_nomT [8 rows = (2j+ch), 128].
  - silu via ScalarE Silu activation (PSUM->SBUF, bias = per-partition AP).
  - PE transpose-back of u_nomT -> natural u_nom planes.
  - Barrier math (dCVaR-CBF + closed-form QP) in natural layout on DVE:
    worst-case GMM mode is analytically the largest-sigma mode (means equal,
    sigma monotone in variance, CVaR coeff > 0), so only one mode is evaluated.
    sqrt via elementwise pow(x, 0.5), projection division via TT divide.
  - Output assembled run-major: partition p holds samples 64p..64p+63 interleaved
    (x,y) -> 512-byte contiguous runs per partition -> efficient store.
"""
import sys

sys.path.insert(0, '/opt/trn_rl_repo')

from contextlib import ExitStack

import numpy as np

import concourse.bass as bass  # noqa: F401
import concourse.tile as tile
from concourse import bacc, mybir
from concourse.bass_utils import run_bass_kernel_spmd
from concourse.masks import make_identity

N_CORES = 8
B = 1_048_576
NF, H1, H2, NC = 16, 128, 32, 2
S = B // N_CORES              # samples per core
SAFE_DIST = 0.8
ALPHA = 2.0
CVAR_COEFF = 1.7549833193248685
SIG_MAX_VAR = 0.3 * 0.3       # largest GMM mode variance (worst-case mode)
EPS_SIG = 1e-8
EPS_DIV = 1e-12

TR = 64                        # samples per partition run
V = 128 * TR                   # natural span = 8192 samples
NBLK = TR * NF // 128          # 8 col-blocks per span
FP32 = mybir.dt.float32

_cached = {}


def build(s_samples=S, n_devices=N_CORES):
    nc = bacc.Bacc("TRN2", target_bir_lowering=False, debug=False,
                   num_devices=n_devices)
    obs_ap = nc.dram_tensor("obs", [s_samples, NF], FP32, kind="ExternalInput").ap()
    w1p0_ap = nc.dram_tensor("w1pad0", [128, 128], FP32, kind="ExternalInput").ap()
    w1p1_ap = nc.dram_tensor("w1pad1", [128, 128], FP32, kind="ExternalInput").ap()
    w2r_ap = nc.dram_tensor("w2rep", [128, 128], FP32, kind="ExternalInput").ap()
    w3b_ap = nc.dram_tensor("w3blk", [128, 8], FP32, kind="ExternalInput").ap()
    b1_ap = nc.dram_tensor("b1c", [128, 1], FP32, kind="ExternalInput").ap()
    b2_ap = nc.dram_tensor("b2rep", [128, 1], FP32, kind="ExternalInput").ap()
    b3_ap = nc.dram_tensor("b3rep", [8, 1], FP32, kind="ExternalInput").ap()
    out_ap = nc.dram_tensor("out", [s_samples, NC], FP32, kind="ExternalOutput").ap()

    with tile.TileContext(nc) as tc, ExitStack() as ctx:
        kernel_body(ctx, tc, out_ap, obs_ap, (w1p0_ap, w1p1_ap), w2r_ap, w3b_ap,
                    b1_ap, b2_ap, b3_ap, s_samples)
    nc.compile()
    return nc


def kernel_body(ctx, tc, out_ap, obs_ap, w1p_aps, w2r_ap, w3b_ap,
                b1_ap, b2_ap, b3_ap, s_samples):
    nc = tc.nc
    nspan = s_samples // V
    span_grp = min(4, nspan)
    SILU = mybir.ActivationFunctionType.Silu
    ALU = mybir.AluOpType

    const = ctx.enter_context(tc.tile_pool(name="const", bufs=1))
    nat_pool = ctx.enter_context(tc.tile_pool(name="nat", bufs=2))
    obsT_pool = ctx.enter_context(tc.tile_pool(name="obsT", bufs=2))
    y1_pool = ctx.enter_context(tc.tile_pool(name="y1", bufs=2))
    y2_pool = ctx.enter_context(tc.tile_pool(name="y2", bufs=2))
    unomT_pool = ctx.enter_context(tc.tile_pool(name="unomT", bufs=2))
    plane_pool = ctx.enter_context(tc.tile_pool(name="plane", bufs=2))
    outb_pool = ctx.enter_context(tc.tile_pool(name="outb", bufs=2))

    ps_tr = ctx.enter_context(tc.tile_pool(name="ps_tr", bufs=1, space="PSUM"))
    ps_y1 = ctx.enter_context(tc.tile_pool(name="ps_y1", bufs=2, space="PSUM"))
    ps_y2 = ctx.enter_context(tc.tile_pool(name="ps_y2", bufs=1, space="PSUM"))
    ps_un = ctx.enter_context(tc.tile_pool(name="ps_un", bufs=1, space="PSUM"))
    ps_t2 = ctx.enter_context(tc.tile_pool(name="ps_t2", bufs=1, space="PSUM"))

    # constants
    w1p0 = const.tile([128, 128], FP32)
    w1p1 = const.tile([128, 128], FP32)
    w2rep = const.tile([128, 128], FP32)
    w3blk = const.tile([128, 8], FP32)
    b1c = const.tile([128, 1], FP32)
    b2rep = const.tile([128, 1], FP32)
    b3rep = const.tile([8, 1], FP32)
    ident = const.tile([128, 128], FP32)
    nc.sync.dma_start(w1p0[:], w1p_aps[0][:])
    nc.sync.dma_start(w1p1[:], w1p_aps[1][:])
    nc.sync.dma_start(w2rep[:], w2r_ap[:])
    nc.sync.dma_start(w3blk[:], w3b_ap[:])
    nc.sync.dma_start(b1c[:], b1_ap[:])
    nc.sync.dma_start(b2rep[:], b2_ap[:])
    nc.sync.dma_start(b3rep[:], b3_ap[:])
    make_identity(nc, ident[:])
    w1pads = (w1p0, w1p1)

    for sg in range(nspan // span_grp):
        PW = span_grp * TR
        relx = plane_pool.tile([128, PW], FP32, tag="relx")
        rely = plane_pool.tile([128, PW], FP32, tag="rely")
        hvx = plane_pool.tile([128, PW], FP32, tag="hvx")
        hvy = plane_pool.tile([128, PW], FP32, tag="hvy")
        unx = plane_pool.tile([128, PW], FP32, tag="unx")
        uny = plane_pool.tile([128, PW], FP32, tag="uny")
        outb = outb_pool.tile([128, 2 * PW], FP32, tag="outb")

        for sl in range(span_grp):
            span = sg * span_grp + sl
            base = span * V
            # ---- natural load: partition p <- samples base+64p .. +63 ----
            obs_nat = nat_pool.tile([128, TR * NF], FP32, tag="obs_nat")
            src = obs_ap[base:base + V, :].rearrange("(p t) f -> p (t f)", p=128)
            nc.sync.dma_start(obs_nat[:], src)

            # ---- barrier input extraction (natural planes) ----
            ob3 = obs_nat[:].rearrange("p (t f) -> p t f", f=NF)
            pl_sl = slice(sl * TR, (sl + 1) * TR)
            nc.vector.tensor_copy(relx[:, pl_sl], ob3[:, :, 6])
            nc.vector.tensor_copy(rely[:, pl_sl], ob3[:, :, 7])
            nc.vector.tensor_copy(hvx[:, pl_sl], ob3[:, :, 8])
            nc.vector.tensor_copy(hvy[:, pl_sl], ob3[:, :, 9])

            # ---- PE transpose natural -> packed obsT (rows 16*t8+f) ----
            obsT = obsT_pool.tile([128, NBLK * 128], FP32, tag="obsT")
            for half in range(2):
                tp = ps_tr.tile([128, 512], FP32, tag="tp")
                for ci in range(4):
                    c = half * 4 + ci
                    nc.tensor.transpose(
                        tp[:, ci * 128:(ci + 1) * 128],
                        obs_nat[:, c * 128:(c + 1) * 128],
                        ident[:])
                nc.vector.tensor_copy(
                    obsT[:, half * 512:(half + 1) * 512], tp[:])

            # layouts: obsT col = c*128 + p (c: col-block, p: partition of span)
            # half h covers c in [4h, 4h+4); within-half col (c4, p).
            # y1sT col = (t8*2 + h)*512 + c4*128 + p
            # y2sT / u_nomT col = (h*2 + sub)*512 + c4*128 + p, groups t8=4*sub+j
            y1sT = y1_pool.tile([128, 8192], FP32, tag="y1sT")
            y2sT = y2_pool.tile([128, 2048], FP32, tag="y2sT")
            unomT = unomT_pool.tile([8, 2048], FP32, tag="unomT")

            PAIRS = ((0, 2), (4, 6), (1, 3), (5, 7))
            for h in range(2):
                hs = slice(h * 512, (h + 1) * 512)
                # ---- L1: one N=512 matmul per group (own PSUM bank) ----
                for pa, pb in PAIRS:
                    y1_ps = ps_y1.tile([128, 1024], FP32, tag="y1T")
                    for slot, t8 in enumerate((pa, pb)):
                        par, s4 = t8 % 2, t8 // 2
                        nc.tensor.matmul(
                            y1_ps[:, slot * 512:(slot + 1) * 512],
                            w1pads[par][32 * s4:32 * s4 + 32, :],
                            obsT[32 * s4:32 * s4 + 32, hs],
                            start=True, stop=True,
                            tile_position=(32 * s4, 0))
                    dst = y1sT[:].rearrange("q (t8 h2 n) -> q t8 h2 n",
                                            t8=8, h2=2)[:, pa:pb + 1:2, h]
                    srcv = y1_ps[:].rearrange("q (s n) -> q s n", s=2)
                    nc.scalar.activation(dst, srcv,
                                         SILU, bias=b1c[:, 0:1], scale=1.0)
                # ---- L2: col-tiled, 4 groups per bank ----
                for sub in range(2):
                    y2T_ps = ps_y2.tile([128, 512], FP32, tag="y2T")
                    for j in range(4):
                        t8 = 4 * sub + j
                        nc.tensor.matmul(
                            y2T_ps[32 * j:32 * j + 32, :],
                            w2rep[:, 32 * j:32 * j + 32],
                            y1sT[:, (t8 * 2 + h) * 512:(t8 * 2 + h + 1) * 512],
                            start=True, stop=True,
                            tile_position=(0, 32 * j))
                    nc.scalar.activation(
                        y2sT[:, (h * 2 + sub) * 512:(h * 2 + sub + 1) * 512],
                        y2T_ps[:], SILU, bias=b2rep[:, 0:1], scale=1.0)
                # ---- L3: blockdiag w3 ----
                for sub in range(2):
                    un_ps = ps_un.tile([8, 512], FP32, tag="unT")
                    qs = slice((h * 2 + sub) * 512, (h * 2 + sub + 1) * 512)
                    nc.tensor.matmul(un_ps[:], w3blk[:], y2sT[:, qs],
                                     start=True, stop=True)
                    nc.vector.tensor_scalar(unomT[:, qs], un_ps[:],
                                            b3rep[:, 0:1], None, ALU.add)

            # ---- T2: transpose-back u_nomT -> natural ----
            t2_ps = ps_t2.tile([128, 128], FP32, tag="tr")
            for k in range(16):
                nc.tensor.transpose(
                    t2_ps[:, k * 8:k * 8 + 8],
                    unomT[:, k * 128:(k + 1) * 128],
                    ident[0:8, 0:8])
            # psum col = 64h+32sub+8c4+2j+ch ; sample t = 32h+8c4+4sub+j
            t2v = t2_ps[:].rearrange("p (h sub c4 j ch) -> p h sub c4 j ch",
                                     h=2, sub=2, c4=4, j=4)
            pxv = unx[:, pl_sl].rearrange("p (h c4 sub j) -> p h sub c4 j",
                                          h=2, c4=4, sub=2)
            pyv = uny[:, pl_sl].rearrange("p (h c4 sub j) -> p h sub c4 j",
                                          h=2, c4=4, sub=2)
            nc.vector.tensor_copy(pxv, t2v[:, :, :, :, :, 0])
            nc.vector.tensor_copy(pyv, t2v[:, :, :, :, :, 1])

        # ================= barrier math (natural, per span-group) ==========
        tmp = plane_pool
        sx = tmp.tile([128, PW], FP32, tag="sx")
        sy = tmp.tile([128, PW], FP32, tag="sy")
        rnsq = tmp.tile([128, PW], FP32, tag="rnsq")
        rdm2 = tmp.tile([128, PW], FP32, tag="rdm2")
        sig = tmp.tile([128, PW], FP32, tag="sig")
        q1 = tmp.tile([128, PW], FP32, tag="q1")
        viol = tmp.tile([128, PW], FP32, tag="viol")
        gnsq = tmp.tile([128, PW], FP32, tag="gnsq")
        coef = tmp.tile([128, PW], FP32, tag="coef")

        V_ = nc.vector
        V_.tensor_mul(sx[:], relx[:], relx[:])
        V_.tensor_mul(sy[:], rely[:], rely[:])
        V_.tensor_add(rnsq[:], sx[:], sy[:])
        V_.tensor_mul(sx[:], hvx[:], relx[:])
        V_.tensor_mul(sy[:], hvy[:], rely[:])
        V_.tensor_add(rdm2[:], sx[:], sy[:])          # rel_dot_mu / 2
        # sigma = sqrt(x), x = 4*var*rnsq + eps_sig
        # rsqrt Newton: seed y0 = bitcast(0x5F3759DF - (i>>1)) built from
        # int<->float convert copies (no shift op needed), 3 NR iterations.
        V_.tensor_scalar(sig[:], rnsq[:], 4.0 * SIG_MAX_VAR, EPS_SIG,
                         ALU.mult, ALU.add)
        yv = coef  # scratch: Newton iterate
        V_.tensor_copy(sx[:], sig[:].bitcast(mybir.dt.int32))   # f = float(i)
        V_.tensor_scalar(sx[:], sx[:], -0.5, 1597463007.0, ALU.mult, ALU.add)
        V_.tensor_copy(yv[:].bitcast(mybir.dt.int32), sx[:])    # y0 bits
        for _ in range(3):
            V_.tensor_mul(sx[:], yv[:], yv[:])
            V_.tensor_mul(sx[:], sx[:], sig[:])
            V_.tensor_scalar(sx[:], sx[:], -0.5, 1.5, ALU.mult, ALU.add)
            V_.tensor_mul(yv[:], yv[:], sx[:])
        V_.tensor_mul(sig[:], sig[:], yv[:])                    # sqrt = x*rsqrt
        # sig <- CVAR*sigma + 2*SAFE^2
        V_.tensor_scalar(sig[:], sig[:], CVAR_COEFF, 2.0 * SAFE_DIST ** 2,
                         ALU.mult, ALU.add)
        # q1 = rdm2 - rnsq - dot(rel, u_nom)
        V_.tensor_sub(q1[:], rdm2[:], rnsq[:])
        V_.tensor_mul(sx[:], relx[:], unx[:])
        V_.tensor_mul(sy[:], rely[:], uny[:])
        V_.tensor_add(sx[:], sx[:], sy[:])
        V_.tensor_sub(q1[:], q1[:], sx[:])
        # viol = 2*q1 + sig
        V_.tensor_scalar(q1[:], q1[:], 2.0, None, ALU.mult)
        V_.tensor_add(viol[:], q1[:], sig[:])
        V_.tensor_scalar(gnsq[:], rnsq[:], 4.0, EPS_DIV, ALU.mult, ALU.add)
        # coef = 2*max(viol,0) * (1/gnsq)
        V_.tensor_scalar(viol[:], viol[:], 0.0, 2.0, ALU.max, ALU.mult)
        V_.reciprocal(gnsq[:], gnsq[:])
        V_.tensor_mul(coef[:], viol[:], gnsq[:])
        V_.tensor_mul(sx[:], coef[:], relx[:])
        V_.tensor_mul(sy[:], coef[:], rely[:])
        ox = outb[:].rearrange("p (w ch) -> p w ch", ch=2)
        V_.tensor_add(ox[:, :, 0], unx[:], sx[:])
        V_.tensor_add(ox[:, :, 1], uny[:], sy[:])

        # ---- store run-major ----
        for sl in range(span_grp):
            span = sg * span_grp + sl
            base = span * V
            dst = out_ap[base:base + V, :].rearrange("(p t) c -> p (t c)", p=128)
            nc.sync.dma_start(dst, outb[:, sl * 2 * TR:(sl + 1) * 2 * TR])


def prep_consts(w1, b1, w2, b2, w3, b3):
    w1pad0 = np.zeros((128, 128), np.float32)
    w1pad1 = np.zeros((128, 128), np.float32)
    w2rep = np.zeros((128, 128), np.float32)
    w3blk = np.zeros((128, 8), np.float32)
    for s4 in range(4):
        w1pad0[32 * s4:32 * s4 + 16, :] = w1.T          # even t8 groups
        w1pad1[32 * s4 + 16:32 * s4 + 32, :] = w1.T     # odd t8 groups
    for j in range(4):
        w2rep[:, 32 * j:32 * j + 32] = w2.T
        w3blk[32 * j:32 * j + 32, 2 * j:2 * j + 2] = w3.T
    return dict(
        w1pad0=w1pad0, w1pad1=w1pad1, w2rep=w2rep, w3blk=w3blk,
        b1c=np.asarray(b1, np.float32).reshape(128, 1),
        b2rep=np.tile(np.asarray(b2, np.float32), 4).reshape(128, 1),
        b3rep=np.tile(np.asarray(b3, np.float32), 4).reshape(8, 1))


def kernel(obs, w1, b1, w2, b2, w3, b3):
    obs = np.asarray(obs, np.float32)
    consts = prep_consts(np.asarray(w1, np.float32), np.asarray(b1, np.float32),
                         np.asarray(w2, np.float32), np.asarray(b2, np.float32),
                         np.asarray(w3, np.float32), np.asarray(b3, np.float32))
    if "nc" not in _cached:
        _cached["nc"] = build()
    nc = _cached["nc"]
    in_maps = []
    for k in range(N_CORES):
        m = {"obs": np.ascontiguousarray(obs[k * S:(k + 1) * S])}
        m.update(consts)
        in_maps.append(m)
    res = run_bass_kernel_spmd(nc, in_maps, list(range(N_CORES)))
    out = np.empty((B, NC), np.float32)
    for k in range(N_CORES):
        out[k * S:(k + 1) * S] = res.results[k]["out"]
    return out



# revision 2
# speedup vs baseline: 1.0108x; 1.0108x over previous
"""BarrierNet Trainium2 kernel v3 (no PSUM tag sharing, FD1024 L1 ACT): 8-core data-parallel Bass/Tile.

Per core (S = 131072 samples), processed in 4 groups of 32768 samples:
  - group load: obs_nat [128, 4096] fp32; partition p holds a 256-sample run.
  - barrier planes (rel/hv) extracted via strided DVE copies (fp32).
  - PE transposes 128x128 blocks of obs_nat -> PSUM; DVE casts to obsT bf16
    (rows 16u+f, u = within-8-block sample, f = feature).
  - L1: per span (8192 samples), 4 rounds of 4 row-band-concurrent bf16
    matmuls (K=32 zero-padded w1 pairs) -> [128,2048] PSUM, drained by one
    FD=2048 Silu activation (bias b1) -> y1sT bf16.
  - L2: col-band-tiled bf16 matmuls (K=128) -> [128,1024] PSUM x2, FD=1024
    Silu (bias b2) -> y2sT bf16.
  - L3: 4 col-tiled matmuls with w3stack placing quarter r's u_nom rows at
    partition band 32r -> one [128,512] PSUM tile; DVE +b3 cast -> u_nomT.
  - T2: 16 PE transposes [8,128] -> natural u_nom planes via strided copies.
  - dCVaR-CBF barrier + closed-form QP in natural fp32 planes on DVE
    (worst-case GMM mode = largest sigma analytically; rsqrt via bitcast
    Newton seed + 2 iterations); output assembled interleaved, run-major
    stores per group-pair.
"""
import sys

sys.path.insert(0, '/opt/trn_rl_repo')

from contextlib import ExitStack

import numpy as np

import concourse.bass as bass  # noqa: F401
import concourse.tile as tile
from concourse import bacc, mybir
from concourse.bass_utils import run_bass_kernel_spmd
from concourse.masks import make_identity

N_CORES = 8
B = 1_048_576
NF, H1, H2, NC = 16, 128, 32, 2
S = B // N_CORES
SAFE_DIST = 0.8
ALPHA = 2.0
CVAR_COEFF = 1.7549833193248685
SIG_MAX_VAR = 0.3 * 0.3
EPS_SIG = 1e-8
EPS_DIV = 1e-12

G = 32768                     # samples per group
NG = S // G                   # 4 groups
V = 8192                      # samples per span
NSP = G // V                  # 4 spans per group
TRUN = 256                    # samples per partition run (per group)
FP32 = mybir.dt.float32
BF16 = mybir.dt.bfloat16

_cached = {}


def build(s_samples=S, n_devices=N_CORES):
    nc = bacc.Bacc("TRN2", target_bir_lowering=False, debug=False,
                   num_devices=n_devices)
    aps = dict(
        obs=nc.dram_tensor("obs", [s_samples, NF], FP32, kind="ExternalInput").ap(),
        w1p0=nc.dram_tensor("w1p0", [128, 128], FP32, kind="ExternalInput").ap(),
        w1p1=nc.dram_tensor("w1p1", [128, 128], FP32, kind="ExternalInput").ap(),
        w2rep=nc.dram_tensor("w2rep", [128, 128], BF16, kind="ExternalInput").ap(),
        w3stk=nc.dram_tensor("w3stk", [128, 128], BF16, kind="ExternalInput").ap(),
        b1c=nc.dram_tensor("b1c", [128, 1], FP32, kind="ExternalInput").ap(),
        b2rep=nc.dram_tensor("b2rep", [128, 1], FP32, kind="ExternalInput").ap(),
        b3stk=nc.dram_tensor("b3stk", [128, 1], FP32, kind="ExternalInput").ap(),
        id8s=nc.dram_tensor("id8s", [128, 8], FP32, kind="ExternalInput").ap(),
    )
    out_ap = nc.dram_tensor("out", [s_samples, NC], FP32, kind="ExternalOutput").ap()
    with tile.TileContext(nc) as tc, ExitStack() as ctx:
        kernel_body(ctx, tc, out_ap, aps)
    nc.compile()
    return nc


def kernel_body(ctx, tc, out_ap, aps):
    nc = tc.nc
    SILU = mybir.ActivationFunctionType.Silu
    ALU = mybir.AluOpType

    const = ctx.enter_context(tc.tile_pool(name="const", bufs=1))
    nat_pool = ctx.enter_context(tc.tile_pool(name="nat", bufs=2))
    obsT_pool = ctx.enter_context(tc.tile_pool(name="obsT", bufs=2))
    y1_pool = ctx.enter_context(tc.tile_pool(name="y1", bufs=2))
    y2_pool = ctx.enter_context(tc.tile_pool(name="y2", bufs=2))
    un_pool = ctx.enter_context(tc.tile_pool(name="un", bufs=2))
    plane_pool = ctx.enter_context(tc.tile_pool(name="plane", bufs=1))

    ps_y1 = ctx.enter_context(tc.tile_pool(name="ps_y1", bufs=2, space="PSUM"))
    ps_tr = ctx.enter_context(tc.tile_pool(name="ps_tr", bufs=2, space="PSUM"))
    ps_un = ctx.enter_context(tc.tile_pool(name="ps_un", bufs=1, space="PSUM"))
    ps_t2 = ctx.enter_context(tc.tile_pool(name="ps_t2", bufs=1, space="PSUM"))

    # ---- constants ----
    w1p0 = const.tile([128, 128], FP32)
    w1p1 = const.tile([128, 128], FP32)
    w2rep = const.tile([128, 128], BF16)
    w3stk = const.tile([128, 128], BF16)
    b1c = const.tile([128, 1], FP32)
    b2rep = const.tile([128, 1], FP32)
    b3stk = const.tile([128, 1], FP32)
    ident = const.tile([128, 128], FP32)
    id8s = const.tile([128, 8], FP32)
    for name, t in (("w1p0", w1p0), ("w1p1", w1p1), ("w2rep", w2rep),
                    ("w3stk", w3stk), ("b1c", b1c), ("b2rep", b2rep),
                    ("b3stk", b3stk), ("id8s", id8s)):
        nc.sync.dma_start(t[:], aps[name][:])
    make_identity(nc, ident[:])
    w1pads = (w1p0, w1p1)

    # ---- whole-core barrier planes ----
    PW = NG * TRUN            # 1024
    relx = plane_pool.tile([128, PW], FP32, tag="relx")
    rely = plane_pool.tile([128, PW], FP32, tag="rely")
    hvx = plane_pool.tile([128, PW], FP32, tag="hvx")
    hvy = plane_pool.tile([128, PW], FP32, tag="hvy")
    unx = plane_pool.tile([128, PW], FP32, tag="unx")
    uny = plane_pool.tile([128, PW], FP32, tag="uny")
    outb = plane_pool.tile([128, 2 * PW], FP32, tag="outb")
    # scratch
    sx = plane_pool.tile([128, PW], FP32, tag="sx")
    sy = plane_pool.tile([128, PW], FP32, tag="sy")
    dx = plane_pool.tile([128, PW], FP32, tag="dx")
    dy = plane_pool.tile([128, PW], FP32, tag="dy")
    rnsq = plane_pool.tile([128, PW], FP32, tag="rnsq")
    sigin = plane_pool.tile([128, PW], FP32, tag="sigin")
    yv = plane_pool.tile([128, PW], FP32, tag="yv")
    viol = plane_pool.tile([128, PW], FP32, tag="viol")

    def barrier_pass(lo, hi):
        """dCVaR-CBF + QP on plane cols [lo:hi) -> outb[:, 2lo:2hi)."""
        V_ = nc.vector
        sl = slice(lo, hi)
        rx, ry = relx[:, sl], rely[:, sl]
        # rnsq = relx^2 + rely^2
        V_.tensor_mul(sx[:, sl], rx, rx)
        V_.tensor_mul(sy[:, sl], ry, ry)
        V_.tensor_add(rnsq[:, sl], sx[:, sl], sy[:, sl])
        # w = (hv - u_nom) . rel - rnsq
        V_.tensor_sub(dx[:, sl], hvx[:, sl], unx[:, sl])
        V_.tensor_sub(dy[:, sl], hvy[:, sl], uny[:, sl])
        V_.tensor_mul(sx[:, sl], dx[:, sl], rx)
        V_.tensor_mul(sy[:, sl], dy[:, sl], ry)
        V_.tensor_add(dx[:, sl], sx[:, sl], sy[:, sl])
        V_.tensor_sub(dx[:, sl], dx[:, sl], rnsq[:, sl])
        # sigin = 4*var*rnsq + eps ; CVAR*sigma via bitcast-Newton rsqrt
        V_.tensor_scalar(sigin[:, sl], rnsq[:, sl], 4.0 * SIG_MAX_VAR, EPS_SIG,
                         ALU.mult, ALU.add)
        V_.tensor_copy(sx[:, sl], sigin[:, sl].bitcast(mybir.dt.int32))
        V_.tensor_scalar(sx[:, sl], sx[:, sl], -0.5, 1597463007.0,
                         ALU.mult, ALU.add)
        V_.tensor_copy(yv[:, sl].bitcast(mybir.dt.int32), sx[:, sl])
        for it in range(2):
            V_.tensor_mul(sx[:, sl], yv[:, sl], yv[:, sl])
            V_.tensor_mul(sx[:, sl], sx[:, sl], sigin[:, sl])
            if it == 0:
                V_.tensor_scalar(sx[:, sl], sx[:, sl], -0.5, 1.5,
                                 ALU.mult, ALU.add)
            else:
                V_.tensor_scalar(sx[:, sl], sx[:, sl], -0.5 * CVAR_COEFF,
                                 1.5 * CVAR_COEFF, ALU.mult, ALU.add)
            V_.tensor_mul(yv[:, sl], yv[:, sl], sx[:, sl])
        V_.tensor_mul(sigin[:, sl], sigin[:, sl], yv[:, sl])  # = CVAR*sigma
        # viol = 2*w + 2*SAFE^2 + CVAR*sigma
        V_.tensor_scalar(dx[:, sl], dx[:, sl], 2.0, 2.0 * SAFE_DIST ** 2,
                         ALU.mult, ALU.add)
        V_.tensor_add(viol[:, sl], dx[:, sl], sigin[:, sl])
        # coef = 2*max(viol,0) / (4*rnsq + eps)
        V_.tensor_scalar(rnsq[:, sl], rnsq[:, sl], 4.0, EPS_DIV,
                         ALU.mult, ALU.add)
        V_.reciprocal(rnsq[:, sl], rnsq[:, sl])
        V_.tensor_scalar(viol[:, sl], viol[:, sl], 0.0, 2.0, ALU.max, ALU.mult)
        V_.tensor_mul(viol[:, sl], viol[:, sl], rnsq[:, sl])
        V_.tensor_mul(sx[:, sl], viol[:, sl], rx)
        V_.tensor_mul(sy[:, sl], viol[:, sl], ry)
        ox = outb[:, 2 * lo:2 * hi].rearrange("p (w c) -> p w c", c=2)
        V_.tensor_add(ox[:, :, 0], unx[:, sl], sx[:, sl])
        V_.tensor_add(ox[:, :, 1], uny[:, sl], sy[:, sl])

    for g in range(NG):
        gbase = g * G
        obs_nat = nat_pool.tile([128, TRUN * NF], FP32, tag="obs_nat")
        src = aps["obs"][gbase:gbase + G, :].rearrange("(p t) f -> p (t f)",
                                                       p=128)
        nc.sync.dma_start(obs_nat[:], src)

        # plane extraction for the whole group
        ob3 = obs_nat[:].rearrange("p (t f) -> p t f", f=NF)
        gsl = slice(g * TRUN, (g + 1) * TRUN)
        nc.vector.tensor_copy(relx[:, gsl], ob3[:, :, 6])
        nc.vector.tensor_copy(rely[:, gsl], ob3[:, :, 7])
        nc.vector.tensor_copy(hvx[:, gsl], ob3[:, :, 8])
        nc.vector.tensor_copy(hvy[:, gsl], ob3[:, :, 9])

        for q in range(NSP):
            # ---- T1: transpose 8 blocks -> obsT bf16 ----
            obsT = obsT_pool.tile([128, 1024], FP32, tag="obsT")
            for fill in range(4):
                tp = ps_tr.tile([128, 256], FP32, tag="tp", name="tp")
                for i in range(2):
                    b = 8 * q + 2 * fill + i
                    nc.tensor.transpose(tp[:, i * 128:(i + 1) * 128],
                                        obs_nat[:, b * 128:(b + 1) * 128],
                                        ident[:])
                nc.vector.tensor_copy(obsT[:, fill * 256:(fill + 1) * 256],
                                      tp[:])

            # ---- L1 ----
            y1sT = y1_pool.tile([128, 8192], BF16, tag="y1sT")
            for par in range(2):
                for h in range(2):
                    r = 2 * par + h
                    for bp in range(2):
                        y1ps = ps_y1.tile([128, 1024], FP32, tag="y1ps")
                        for e in range(2):
                            s4 = 2 * bp + e
                            nc.tensor.matmul(
                                y1ps[:, e * 512:(e + 1) * 512],
                                w1pads[par][32 * s4:32 * s4 + 32, :],
                                obsT[32 * s4:32 * s4 + 32,
                                     512 * h:512 * h + 512],
                                start=True, stop=True,
                                tile_position=(32 * s4, 0))
                        nc.scalar.activation(
                            y1sT[:, r * 2048 + bp * 1024:
                                 r * 2048 + bp * 1024 + 1024],
                            y1ps[:], SILU, bias=b1c[:, 0:1], scale=1.0)

            # ---- L2 ----
            y2sT = y2_pool.tile([128, 2048], BF16, tag="y2sT")
            for t in range(2):
                y2ps = ps_y1.tile([128, 1024], FP32, tag="y1ps", name="y2ps")
                for jj in range(4):
                    u = 4 * t + jj
                    for h in range(2):
                        nc.tensor.matmul(
                            y2ps[32 * jj:32 * jj + 32, 512 * h:512 * h + 512],
                            w2rep[:, 32 * jj:32 * jj + 32],
                            y1sT[:, (2 * (u % 2) + h) * 2048 + 512 * (u // 2):
                                 (2 * (u % 2) + h) * 2048 + 512 * (u // 2) + 512],
                            start=True, stop=True,
                            tile_position=(0, 32 * jj))
                nc.scalar.activation(y2sT[:, t * 1024:(t + 1) * 1024],
                                     y2ps[:], SILU, bias=b2rep[:, 0:1],
                                     scale=1.0)

            # ---- L3: quarters -> partition bands ----
            unps = ps_un.tile([128, 512], FP32, tag="unps", name="unps")
            for r in range(4):
                nc.tensor.matmul(unps[32 * r:32 * r + 8, :],
                                 w3stk[:, 32 * r:32 * r + 8],
                                 y2sT[:, 512 * r:512 * r + 512],
                                 start=True, stop=True,
                                 tile_position=(0, 32 * r))
            unomT = un_pool.tile([128, 512], FP32, tag="unomT")
            nc.vector.tensor_scalar(unomT[:], unps[:], b3stk[:, 0:1], None,
                                    ALU.add)

            # ---- T2: 4 full-block transposes [128,128] -> natural ----
            t2p = ps_t2.tile([128, 512], FP32, tag="t2p", name="t2p")
            for k in range(4):
                nc.tensor.transpose(t2p[:, k * 128:(k + 1) * 128],
                                    unomT[:, 128 * k:128 * k + 128],
                                    ident[:])
            # t2 col = 128k + 64T + 32h + (2jj+ch, in low 8 of each 32)
            # t = 64q + 32h + 8k + 4T + jj
            t2v5 = t2p[:].rearrange("p (k T h m) -> p T h k m", k=4, T=2, h=2)
            t2v = t2v5[:, :, :, :, 0:8].rearrange(
                "p T h k (jj ch) -> p T h k jj ch", ch=2)
            base = g * TRUN + 64 * q
            pxv = unx[:, base:base + 64].rearrange(
                "p (h k T jj) -> p T h k jj", h=2, k=4, T=2)
            pyv = uny[:, base:base + 64].rearrange(
                "p (h k T jj) -> p T h k jj", h=2, k=4, T=2)
            nc.vector.tensor_copy(pxv, t2v[:, :, :, :, :, 0])
            nc.vector.tensor_copy(pyv, t2v[:, :, :, :, :, 1])

        if g % 2 == 1:
            lo = (g - 1) * TRUN
            hi = (g + 1) * TRUN
            barrier_pass(lo, hi)
            for gg in (g - 1, g):
                dst = out_ap[gg * G:(gg + 1) * G, :].rearrange(
                    "(p t) c -> p (t c)", p=128)
                nc.sync.dma_start(dst, outb[:, gg * 2 * TRUN:(gg + 1) * 2 * TRUN])


def prep_consts(w1, b1, w2, b2, w3, b3):
    w1p0 = np.zeros((128, 128), np.float32)
    w1p1 = np.zeros((128, 128), np.float32)
    w2rep = np.zeros((128, 128), np.float32)
    w3stk = np.zeros((128, 128), np.float32)
    b3stk = np.zeros((128, 1), np.float32)
    id8s = np.zeros((128, 8), np.float32)
    for r in range(4):
        for i in range(8):
            id8s[32 * r + i, i] = 1.0
    for s4 in range(4):
        w1p0[32 * s4:32 * s4 + 16, :] = w1.T
        w1p1[32 * s4 + 16:32 * s4 + 32, :] = w1.T
    for jj in range(4):
        w2rep[:, 32 * jj:32 * jj + 32] = w2.T
    for r in range(4):
        for jj in range(4):
            for ch in range(2):
                w3stk[32 * jj:32 * jj + 32, 32 * r + 2 * jj + ch] = w3[ch, :]
                b3stk[32 * r + 2 * jj + ch, 0] = b3[ch]
    return dict(
        w1p0=w1p0, w1p1=w1p1, w2rep=_to_bf16(w2rep), w3stk=_to_bf16(w3stk),
        id8s=id8s,
        b1c=np.asarray(b1, np.float32).reshape(128, 1),
        b2rep=np.tile(np.asarray(b2, np.float32), 4).reshape(128, 1),
        b3stk=b3stk)


def _to_bf16(a):
    import jax.numpy as jnp
    return np.asarray(jnp.asarray(a, jnp.bfloat16))


def kernel(obs, w1, b1, w2, b2, w3, b3):
    obs = np.asarray(obs, np.float32)
    consts = prep_consts(np.asarray(w1, np.float32), np.asarray(b1, np.float32),
                         np.asarray(w2, np.float32), np.asarray(b2, np.float32),
                         np.asarray(w3, np.float32), np.asarray(b3, np.float32))
    if "nc" not in _cached:
        _cached["nc"] = build()
    nc = _cached["nc"]
    in_maps = []
    for k in range(N_CORES):
        m = {"obs": np.ascontiguousarray(obs[k * S:(k + 1) * S])}
        m.update(consts)
        in_maps.append(m)
    res = run_bass_kernel_spmd(nc, in_maps, list(range(N_CORES)))
    out = np.empty((B, NC), np.float32)
    for k in range(N_CORES):
        out[k * S:(k + 1) * S] = res.results[k]["out"]
    return out
